# revision 6
# baseline (speedup 1.0000x reference)
"""Self-contained Trainium2 Bass kernel for nn_Decoder_79809082294812.

kernel(**inputs) takes the FULL unsharded inputs (embeddings [1024,1000,128],
remaining_capacity [1024], Wqg [257,128], Wkg/Wvg/Wog/Wqo/Wko [128,128],
current_node [1024], mask [1024,1000]) and returns (probs, logits), each
[1024, 1000] float32 - matching the reference decoder.

Sharding: pure data-parallel over the batch dim across 8 NeuronCores
(128 batch elements per core); weights replicated.

v3 design ("all-flipped, host-assisted, software-pipelined"):
  * The whole O(B*E^2) pre-attention chain (graph mean, context, q, U) is
    computed on the host; the device receives per-element glimpse key
    vectors U_b [E, H] directly.
  * compat is computed TRANSPOSED on-chip: for each (element i, n-chunk c)
    the stationary operand is an embT chunk [128e, 125n] and the moving
    operand is U_i [128, 8].  The psum result [125n, (c,i,h)] is already in
    "attn^T" orientation, so the PE transposes / psum->sbuf copies / attn
    normalization of the old kernel disappear.  No max subtraction before
    exp (|compat| < ~7).
  * The attn normalization (1/rowsum) is applied at the [128,8] heads
    stage via a tiny matmul-built broadcast.
  * comp (pre-tanh logits) is computed transposed too ([125n, (i,c)]
    psum), PE-transposed once per group, DMAed out as one contiguous
    block.  tanh / softmax / logits run on the host.
  * Embeddings ship in both layouts (nat + embT) as bf16, host-packed in
    group-blob layout so every DMA is a contiguous ~[128, 16KB] transfer.
    All constants ride in two single-DMA blobs ahead of the bulk.
  * Two-stage software pipeline: group g's post-attention chain (recip,
    heads, w, comp) is emitted after group g+1's compat so the PE queue
    always has independent work while DVE/ACT handle the serial steps.
"""
import contextlib
import ctypes
import math
import os
import sys
import types

sys.path.insert(0, '/opt/trn_rl_repo')

from contextlib import ExitStack
import numpy as np
import ml_dtypes

import concourse.bass as bass
import concourse.tile as tile
from concourse import bacc, mybir
from concourse.bass_utils import run_bass_kernel_spmd

F32 = mybir.dt.float32
BF16 = mybir.dt.bfloat16
AF = mybir.ActivationFunctionType
AX = mybir.AxisListType
ALU = mybir.AluOpType
BF16_NP = ml_dtypes.bfloat16

B = 1024
N = 1000
E = 128
H = 8
D = 16
N_CORES = 8
BC = B // N_CORES   # batch elements per core
G = 8               # elements per group
NG = BC // G        # 16 groups
NCH = 8             # n-chunks per element (chunk c = nodes [125c, 125c+125))
CH = 125            # chunk size
PF = 3              # DMA prefetch depth (groups)

# f32 const blob column layout
F_IDENT = 0      # [125,125] identity (for the output transpose)
F_WVG = 128      # [128,128]
F_WBIG = 256     # [128,128]
F_M8 = 384       # [128,64]
F_MASK64 = 448   # [64,128]
F_IMASK = 576    # [64,8]
F_ONEF = 584     # [1,1]
F_COLS = 585
# bf16 const blob: ug [128, BC*H] then ones column
U_COLS = BC * H
B_COLS = U_COLS + 1

_NC_CACHE = {}
LAST_RESULT = None   # BassKernelResults of the most recent run (for profiling)


# --------------------------------------------------------------------------
# Optional NTFF profiling hook (enabled only when BASS_TRACE is set).
# --------------------------------------------------------------------------
def _install_profile_shim():
    so_path = '/opt/axon/libaxon_pjrt.so'
    try:
        import antenv
    except ImportError:
        return
    if 'antenv.axon_hooks' not in sys.modules:
        mod = types.ModuleType('antenv.axon_hooks')
        mod._hook = None

        def set_axon_ntff_profile_hook(h):
            mod._hook = h

        def get_axon_ntff_profile_hook():
            return mod._hook

        mod.set_axon_ntff_profile_hook = set_axon_ntff_profile_hook
        mod.get_axon_ntff_profile_hook = get_axon_ntff_profile_hook
        sys.modules['antenv.axon_hooks'] = mod
        antenv.axon_hooks = mod
    mod = sys.modules['antenv.axon_hooks']
    if mod.get_axon_ntff_profile_hook() is not None:
        return
    try:
        lib = ctypes.CDLL(so_path)
    except OSError:
        return
    if not hasattr(lib, "axon_start_nrt_profile"):
        return
    lib.axon_start_nrt_profile.argtypes = [ctypes.POINTER(ctypes.c_int64),
                                           ctypes.c_size_t]
    lib.axon_start_nrt_profile.restype = ctypes.c_int64
    lib.axon_stop_nrt_profile.argtypes = [ctypes.c_char_p]
    lib.axon_stop_nrt_profile.restype = ctypes.c_int64

    @contextlib.contextmanager
    def _hook(output_dir, device_ids):
        import jax
        jax.devices()
        if device_ids:
            ids = (ctypes.c_int64 * len(device_ids))(*device_ids)
            rc = lib.axon_start_nrt_profile(ids, len(device_ids))
        else:
            rc = lib.axon_start_nrt_profile(None, 0)
        if rc != 0:
            raise RuntimeError(f"axon_start_nrt_profile rc={rc}")
        try:
            yield
        finally:
            n = lib.axon_stop_nrt_profile(str(output_dir).encode())
            if n < 0:
                raise RuntimeError(f"axon_stop_nrt_profile rc={n}")

    mod.set_axon_ntff_profile_hook(_hook)
    import concourse.bass_utils as bu
    bu.upload_artifacts = lambda tmpdir: f"local:{tmpdir}"


def _build_nc(n_devices=N_CORES):
    nc = bacc.Bacc("TRN2", target_bir_lowering=False, debug=False,
                   num_devices=n_devices)

    natg = nc.dram_tensor("natg", [NG, CH, G, NCH, E], BF16,
                          kind="ExternalInput").ap()
    embTg = nc.dram_tensor("embTg", [NG, E, G, NCH, CH], BF16,
                           kind="ExternalInput").ap()
    cf_d = nc.dram_tensor("cf", [E, F_COLS], F32, kind="ExternalInput").ap()
    cb_d = nc.dram_tensor("cb", [E, B_COLS], BF16, kind="ExternalInput").ap()
    outc = nc.dram_tensor("outc", [NG, G * NCH, CH], F32,
                          kind="ExternalOutput").ap()

    with tile.TileContext(nc) as tc, ExitStack() as ctx:
        # ---- constants: two blob DMAs on the (16-engine) gpsimd queue ----
        cpool = ctx.enter_context(tc.tile_pool(name="consts", bufs=1))
        cb = cpool.tile([E, B_COLS], BF16, tag="cb")
        nc.gpsimd.dma_start(cb[:], cb_d[:])
        cf = cpool.tile([E, F_COLS], F32, tag="cf")
        nc.gpsimd.dma_start(cf[:], cf_d[:])

        ug = cb[:, 0:U_COLS]
        ones1 = cb[:CH, U_COLS:U_COLS + 1]
        identf = cf[:CH, F_IDENT:F_IDENT + CH]
        wvg = cf[:, F_WVG:F_WVG + E]
        wbig = cf[:, F_WBIG:F_WBIG + E]
        m8rep = cf[:, F_M8:F_M8 + G * H]
        mask64 = cf[:G * H, F_MASK64:F_MASK64 + E]
        imask = cf[:G * H, F_IMASK:F_IMASK + G]
        onef = cf[:1, F_ONEF:F_ONEF + 1]

        # ---- pools ----
        nat_pool = ctx.enter_context(tc.tile_pool(name="nat", bufs=PF + 2))
        embT_pool = ctx.enter_context(tc.tile_pool(name="embT", bufs=PF + 2))
        exp_pool = ctx.enter_context(tc.tile_pool(name="exp", bufs=2))
        sm_pool = ctx.enter_context(tc.tile_pool(name="smalls", bufs=3))
        out_pool = ctx.enter_context(tc.tile_pool(name="outs", bufs=2))

        # PSUM (8 banks): P1 x2 + pA x2 + P3 x2 + smalls x2
        p1_pool = ctx.enter_context(tc.tile_pool(name="p1", bufs=2, space="PSUM"))
        pa_pool = ctx.enter_context(tc.tile_pool(name="pa", bufs=2, space="PSUM"))
        p3_pool = ctx.enter_context(tc.tile_pool(name="p3", bufs=2, space="PSUM"))
        ps_pool = ctx.enter_context(tc.tile_pool(name="ps", bufs=2, space="PSUM"))

        tiles = {}    # g -> (natsb, embTsb)
        state = {}    # g -> dict of live tiles

        def issue_load(g):
            eb = embT_pool.tile([E, G, NCH, CH], BF16, tag="embT")
            nc.gpsimd.dma_start(eb[:], embTg[g])
            nb = nat_pool.tile([CH, G, NCH, E], BF16, tag="nat")
            nc.gpsimd.dma_start(nb[:], natg[g])
            tiles[g] = (nb, eb)

        def emit_compat(g):
            natsb, embTsb = tiles.pop(g)
            P1 = p1_pool.tile([CH, G * H * NCH], F32, tag="p1")
            for i in range(G):
                u_sl = ug[:, (G * g + i) * H:(G * g + i + 1) * H]
                for c in range(NCH):
                    nc.tensor.matmul(
                        P1[:, G * H * c + H * i: G * H * c + H * i + H],
                        embTsb[:, i, c, :], u_sl, start=True, stop=True)
            expT = exp_pool.tile([CH, G * H * NCH], BF16, tag="expT")
            nc.scalar.activation(expT[:], P1[:], AF.Exp)
            state[g] = dict(natsb=natsb, embTsb=embTsb, expT=expT)

        def emit_phaseA(g):
            st = state[g]
            expT, natsb = st["expT"], st["natsb"]
            sums_p = ps_pool.tile([1, G * H * NCH], F32, tag="ps")
            nc.tensor.matmul(sums_p[:], ones1, expT[:], start=True, stop=True)
            pA = pa_pool.tile([E, G * H], F32, tag="pa")
            for i in range(G):
                for c in range(NCH):
                    nc.tensor.matmul(
                        pA[:, H * i: H * i + H],
                        natsb[:, i, c, :],
                        expT[:, G * H * c + H * i: G * H * c + H * i + H],
                        start=(c == 0), stop=(c == NCH - 1))
            # row sums -> reciprocal (DVE; runs under the next group's PE work)
            s64 = sm_pool.tile([1, G * H], F32, tag="s64")
            nc.vector.tensor_reduce(
                s64[:], sums_p[:].rearrange("p (c i h) -> p (i h) c",
                                            c=NCH, i=G),
                axis=AX.X, op=ALU.add)
            r64 = sm_pool.tile([1, G * H], F32, tag="r64")
            nc.vector.reciprocal(r64[:], s64[:])
            st["pA"] = pA
            st["r64"] = r64

        def emit_phaseB(g):
            st = state[g]
            # R[hd, i] = r64[(i, hd//16)] via two tiny matmuls
            rT_p = ps_pool.tile([G * H, 1], F32, tag="ps")
            nc.tensor.matmul(rT_p[:], st["r64"][:], onef, start=True, stop=True)
            rT = sm_pool.tile([G * H, 1], F32, tag="rT")
            nc.vector.tensor_scalar_add(rT[:], rT_p[:], 0.0)
            rmat = sm_pool.tile([G * H, G], F32, tag="rmat")
            nc.vector.tensor_scalar_mul(rmat[:], imask, rT[:])
            R_p = ps_pool.tile([E, G], F32, tag="ps")
            nc.tensor.matmul(R_p[:], mask64, rmat[:], start=True, stop=True)
            R_sb = sm_pool.tile([E, G], F32, tag="R")
            nc.vector.tensor_scalar_add(R_sb[:], R_p[:], 0.0)

            # heads = extract(Wvg.T @ A) * R;  w = Wbig.T @ heads
            A_sb = sm_pool.tile([E, G * H], F32, tag="A")
            nc.vector.tensor_scalar_add(A_sb[:], st["pA"][:], 0.0)
            pheads = ps_pool.tile([E, G * H], F32, tag="ps")
            nc.tensor.matmul(pheads[:], wvg, A_sb[:], start=True, stop=True)
            tmp = sm_pool.tile([E, G * H], F32, tag="tmp")
            nc.vector.tensor_mul(tmp[:], pheads[:], m8rep)
            heads8 = sm_pool.tile([E, G], F32, tag="heads8")
            nc.vector.reduce_sum(
                heads8[:], tmp[:].rearrange("p (i h) -> p i h", h=H),
                axis=AX.X)
            heads8r = sm_pool.tile([E, G], F32, tag="heads8r")
            nc.vector.tensor_mul(heads8r[:], heads8[:], R_sb[:])
            pw = ps_pool.tile([E, G], F32, tag="ps")
            nc.tensor.matmul(pw[:], wbig, heads8r[:], start=True, stop=True)
            w8 = sm_pool.tile([E, G], BF16, tag="w8")
            nc.vector.tensor_scalar_add(w8[:], pw[:], 0.0)
            st["w8"] = w8

        def emit_phaseC(g):
            st = state.pop(g)
            embTsb, w8 = st["embTsb"], st["w8"]
            P3 = p3_pool.tile([CH, G * NCH], F32, tag="p3")
            for i in range(G):
                for c in range(NCH):
                    nc.tensor.matmul(P3[:, NCH * i + c: NCH * i + c + 1],
                                     embTsb[:, i, c, :], w8[:, i:i + 1],
                                     start=True, stop=True)
            c64 = sm_pool.tile([CH, G * NCH], F32, tag="c64")
            nc.vector.tensor_scalar_add(c64[:], P3[:], 0.0)
            pt = ps_pool.tile([G * NCH, CH], F32, tag="ps")
            nc.tensor.transpose(pt[:], c64[:], identf)
            outsb = out_pool.tile([G * NCH, CH], F32, tag="outsb")
            nc.vector.tensor_scalar_add(outsb[:], pt[:], 0.0)
            nc.sync.dma_start(outc[g], outsb[:])

        for g in range(min(PF, NG)):
            issue_load(g)

        for g in range(NG):
            emit_compat(g)
            if g > 0:
                emit_phaseB(g - 1)
                emit_phaseC(g - 1)
            if g + PF < NG:
                issue_load(g + PF)
            emit_phaseA(g)
        emit_phaseB(NG - 1)
        emit_phaseC(NG - 1)

    nc.compile()
    return nc


def _get_nc():
    key = N_CORES
    if key not in _NC_CACHE:
        _NC_CACHE[key] = _build_nc(key)
    return _NC_CACHE[key]


def _make_const_blobs(Wvg, wbig):
    cf = np.zeros((E, F_COLS), np.float32)
    cf[:CH, F_IDENT:F_IDENT + CH] = np.eye(CH, dtype=np.float32)
    cf[:, F_WVG:F_WVG + E] = Wvg
    cf[:, F_WBIG:F_WBIG + E] = wbig
    m8 = np.zeros((E, G, H), np.float32)
    for hd in range(E):
        m8[hd, :, hd // D] = 1.0
    cf[:, F_M8:F_M8 + G * H] = m8.reshape(E, G * H)
    mk = np.zeros((G, H, E), np.float32)
    for hd in range(E):
        mk[:, hd // D, hd] = 1.0
    cf[:G * H, F_MASK64:F_MASK64 + E] = mk.reshape(G * H, E)
    im = np.zeros((G, H, G), np.float32)
    for i in range(G):
        im[i, :, i] = 1.0
    cf[:G * H, F_IMASK:F_IMASK + G] = im.reshape(G * H, G)
    cf[0, F_ONEF] = 1.0
    return np.ascontiguousarray(cf)


def kernel(embeddings, remaining_capacity, Wqg, Wkg, Wvg, Wog, Wqo, Wko,
           current_node, mask):
    global LAST_RESULT
    embeddings = np.asarray(embeddings, dtype=np.float32)
    remaining_capacity = np.asarray(remaining_capacity, dtype=np.float32)
    Wqg = np.asarray(Wqg, dtype=np.float32)
    Wkg = np.asarray(Wkg, dtype=np.float32)
    Wvg = np.asarray(Wvg, dtype=np.float32)
    Wog = np.asarray(Wog, dtype=np.float32)
    Wqo = np.asarray(Wqo, dtype=np.float32)
    Wko = np.asarray(Wko, dtype=np.float32)
    current_node = np.asarray(current_node)
    mask = np.asarray(mask)
    assert embeddings.shape == (B, N, E)

    trace = bool(os.environ.get("BASS_TRACE"))
    if trace:
        _install_profile_shim()

    # ---- host-side pre-attention chain ----
    graph = embeddings.mean(axis=1)                                # [B, E]
    cur = embeddings[np.arange(B), current_node.astype(np.int64)]  # [B, E]
    context = np.concatenate(
        [graph, cur, remaining_capacity[:, None]], axis=-1)        # [B, 257]
    q = (context @ Wqg).reshape(B, H, D)
    U = np.einsum('ehd,bhd->beh', Wkg.reshape(E, H, D), q) / math.sqrt(D)

    cf = _make_const_blobs(
        np.ascontiguousarray(Wvg).astype(np.float32),
        np.ascontiguousarray((Wog @ Wqo @ Wko.T) / math.sqrt(E)).astype(
            np.float32))

    emb_bf = embeddings.astype(BF16_NP)

    nc = _get_nc()
    in_maps = []
    for c in range(N_CORES):
        sl = slice(c * BC, (c + 1) * BC)
        t = emb_bf[sl].reshape(NG, G, NCH, CH, E)
        natg = np.ascontiguousarray(t.transpose(0, 3, 1, 2, 4))
        embTg = np.ascontiguousarray(t.transpose(0, 4, 1, 2, 3))
        cb = np.zeros((E, B_COLS), BF16_NP)
        cb[:, :U_COLS] = U[sl].transpose(1, 0, 2).reshape(E, BC * H)
        cb[:CH, U_COLS] = 1.0
        m = {"natg": natg, "embTg": embTg, "cf": cf,
             "cb": np.ascontiguousarray(cb)}
        in_maps.append(m)

    kw = {}
    if trace:
        kw = dict(trace=True, trace_cores=[0])
    res = run_bass_kernel_spmd(nc, in_maps, list(range(N_CORES)), **kw)
    LAST_RESULT = res

    comp = np.concatenate(
        [res.results[c]["outc"].reshape(BC, N) for c in range(N_CORES)],
        axis=0)

    logits = (10.0 * np.tanh(comp)).astype(np.float32)
    mx = logits.max(axis=-1, keepdims=True)
    ex = np.exp(logits - mx)
    probs = (ex / ex.sum(axis=-1, keepdims=True)).astype(np.float32)

    if mask.any():
        # General-correctness slow path (the spec always sends an all-False
        # mask): the mask affects the glimpse attention too, so recompute
        # everything for the masked rows on the host.
        probs, logits = _numpy_full(embeddings, remaining_capacity, Wqg, Wkg,
                                    Wvg, Wog, Wqo, Wko, cur, mask)

    return probs.astype(np.float32), logits.astype(np.float32)


def _numpy_full(emb, capv, Wqg, Wkg, Wvg, Wog, Wqo, Wko, cur, mask):
    graph = emb.mean(axis=1)
    context = np.concatenate([graph, cur, capv[:, None]], axis=-1)
    q = (context @ Wqg).reshape(B, H, D)
    k = (emb @ Wkg).reshape(B, N, H, D)
    v = (emb @ Wvg).reshape(B, N, H, D)
    compat = np.einsum('bhd,bnhd->bhn', q, k) / math.sqrt(D)
    compat = np.where(mask[:, None, :], -np.inf, compat)
    m = compat.max(axis=-1, keepdims=True)
    a = np.exp(compat - m)
    attn = a / a.sum(axis=-1, keepdims=True)
    heads = np.einsum('bhn,bnhd->bhd', attn, v).reshape(B, E)
    glimpse = heads @ Wog
    qo = glimpse @ Wqo
    ko = emb @ Wko
    comp = np.einsum('be,bne->bn', qo, ko) / math.sqrt(E)
    logits = 10.0 * np.tanh(comp)
    logits = np.where(mask, -np.inf, logits)
    m2 = logits.max(axis=-1, keepdims=True)
    a2 = np.exp(logits - m2)
    probs = a2 / a2.sum(axis=-1, keepdims=True)
    return probs.astype(np.float32), logits.astype(np.float32)


# revision 9
# speedup vs baseline: 1.7492x; 1.7492x over previous
"""Self-contained Trainium2 Bass kernel for nn_Decoder_79809082294812.

kernel(**inputs) takes the FULL unsharded inputs (embeddings [1024,1000,128],
remaining_capacity [1024], Wqg [257,128], Wkg/Wvg/Wog/Wqo/Wko [128,128],
current_node [1024], mask [1024,1000]) and returns (probs, logits), each
[1024, 1000] float32 - matching the reference decoder.

Sharding: pure data-parallel over the batch dim across 8 NeuronCores
(128 batch elements per core); weights replicated.

v3 design ("all-flipped, host-assisted, software-pipelined"):
  * The whole O(B*E^2) pre-attention chain (graph mean, context, q, U) is
    computed on the host; the device receives per-element glimpse key
    vectors U_b [E, H] directly.
  * compat is computed TRANSPOSED on-chip: for each (element i, n-chunk c)
    the stationary operand is an embT chunk [128e, 125n] and the moving
    operand is U_i [128, 8].  The psum result [125n, (c,i,h)] is already in
    "attn^T" orientation, so the PE transposes / psum->sbuf copies / attn
    normalization of the old kernel disappear.  No max subtraction before
    exp (|compat| < ~7).
  * The attn normalization (1/rowsum) is applied at the [128,8] heads
    stage via a tiny matmul-built broadcast.
  * comp (pre-tanh logits) is computed transposed too ([125n, (i,c)]
    psum), PE-transposed once per group, DMAed out as one contiguous
    block.  tanh / softmax / logits run on the host.
  * Embeddings ship in both layouts (nat + embT) as bf16, host-packed in
    group-blob layout so every DMA is a contiguous ~[128, 16KB] transfer.
    All constants ride in two single-DMA blobs ahead of the bulk.
  * Two-stage software pipeline: group g's post-attention chain (recip,
    heads, w, comp) is emitted after group g+1's compat so the PE queue
    always has independent work while DVE/ACT handle the serial steps.
"""
import contextlib
import ctypes
import math
import os
import sys
import types

sys.path.insert(0, '/opt/trn_rl_repo')

from contextlib import ExitStack
import numpy as np
import ml_dtypes

import concourse.bass as bass
import concourse.tile as tile
from concourse import bacc, mybir
from concourse.bass_utils import run_bass_kernel_spmd

F32 = mybir.dt.float32
BF16 = mybir.dt.bfloat16
AF = mybir.ActivationFunctionType
AX = mybir.AxisListType
ALU = mybir.AluOpType
BF16_NP = ml_dtypes.bfloat16

B = 1024
N = 1000
E = 128
H = 8
D = 16
N_CORES = 8
BC = B // N_CORES   # batch elements per core
G = 8               # elements per group
NG = BC // G        # 16 groups
NCH = 8             # n-chunks per element (chunk c = nodes [128c, 128c+128))
CH = 128            # chunk size (last chunk: 104 valid nodes + 24 zero pad)
NPAD = NCH * CH     # 1024
PF = 4              # DMA prefetch depth (groups)

# f32 const blob column layout
F_IDENT = 0      # [128,128] identity (for the output transpose)
F_WVG = 128      # [128,128]
F_WBIG = 256     # [128,128]
F_M8 = 384       # [128,64]
F_MASK64 = 448   # [64,128]
F_IMASK = 576    # [64,8]
F_ONEF = 584     # [1,1]
F_COLS = 585
# bf16 const blob: ug [128, BC*H] then ones column
U_COLS = BC * H
B_COLS = U_COLS + 1

_NC_CACHE = {}
LAST_RESULT = None   # BassKernelResults of the most recent run (for profiling)


# --------------------------------------------------------------------------
# Optional NTFF profiling hook (enabled only when BASS_TRACE is set).
# --------------------------------------------------------------------------
def _install_profile_shim():
    so_path = '/opt/axon/libaxon_pjrt.so'
    try:
        import antenv
    except ImportError:
        return
    if 'antenv.axon_hooks' not in sys.modules:
        mod = types.ModuleType('antenv.axon_hooks')
        mod._hook = None

        def set_axon_ntff_profile_hook(h):
            mod._hook = h

        def get_axon_ntff_profile_hook():
            return mod._hook

        mod.set_axon_ntff_profile_hook = set_axon_ntff_profile_hook
        mod.get_axon_ntff_profile_hook = get_axon_ntff_profile_hook
        sys.modules['antenv.axon_hooks'] = mod
        antenv.axon_hooks = mod
    mod = sys.modules['antenv.axon_hooks']
    if mod.get_axon_ntff_profile_hook() is not None:
        return
    try:
        lib = ctypes.CDLL(so_path)
    except OSError:
        return
    if not hasattr(lib, "axon_start_nrt_profile"):
        return
    lib.axon_start_nrt_profile.argtypes = [ctypes.POINTER(ctypes.c_int64),
                                           ctypes.c_size_t]
    lib.axon_start_nrt_profile.restype = ctypes.c_int64
    lib.axon_stop_nrt_profile.argtypes = [ctypes.c_char_p]
    lib.axon_stop_nrt_profile.restype = ctypes.c_int64

    @contextlib.contextmanager
    def _hook(output_dir, device_ids):
        import jax
        jax.devices()
        if device_ids:
            ids = (ctypes.c_int64 * len(device_ids))(*device_ids)
            rc = lib.axon_start_nrt_profile(ids, len(device_ids))
        else:
            rc = lib.axon_start_nrt_profile(None, 0)
        if rc != 0:
            raise RuntimeError(f"axon_start_nrt_profile rc={rc}")
        try:
            yield
        finally:
            n = lib.axon_stop_nrt_profile(str(output_dir).encode())
            if n < 0:
                raise RuntimeError(f"axon_stop_nrt_profile rc={n}")

    mod.set_axon_ntff_profile_hook(_hook)
    import concourse.bass_utils as bu
    bu.upload_artifacts = lambda tmpdir: f"local:{tmpdir}"


def _build_nc(n_devices=N_CORES):
    nc = bacc.Bacc("TRN2", target_bir_lowering=False, debug=False,
                   num_devices=n_devices)

    natg = nc.dram_tensor("natg", [NG, CH, G, NCH, E], BF16,
                          kind="ExternalInput").ap()
    embTg = nc.dram_tensor("embTg", [NG, E, G, NCH, CH], BF16,
                           kind="ExternalInput").ap()
    cf_d = nc.dram_tensor("cf", [E, F_COLS], F32, kind="ExternalInput").ap()
    cb_d = nc.dram_tensor("cb", [E, B_COLS], BF16, kind="ExternalInput").ap()
    outc = nc.dram_tensor("outc", [NG, G * NCH, CH], F32,
                          kind="ExternalOutput").ap()

    with tile.TileContext(nc) as tc, ExitStack() as ctx:
        # ---- constants: two blob DMAs on the (16-engine) gpsimd queue ----
        cpool = ctx.enter_context(tc.tile_pool(name="consts", bufs=1))
        cb = cpool.tile([E, B_COLS], BF16, tag="cb")
        nc.gpsimd.dma_start(cb[:], cb_d[:])
        cf = cpool.tile([E, F_COLS], F32, tag="cf")
        nc.gpsimd.dma_start(cf[:], cf_d[:])

        ug = cb[:, 0:U_COLS]
        ones1 = cb[:CH, U_COLS:U_COLS + 1]
        identf = cf[:CH, F_IDENT:F_IDENT + CH]
        wvg = cf[:, F_WVG:F_WVG + E]
        wbig = cf[:, F_WBIG:F_WBIG + E]
        m8rep = cf[:, F_M8:F_M8 + G * H]
        mask64 = cf[:G * H, F_MASK64:F_MASK64 + E]
        imask = cf[:G * H, F_IMASK:F_IMASK + G]
        onef = cf[:1, F_ONEF:F_ONEF + 1]

        # ---- pools ----
        nat_pool = ctx.enter_context(tc.tile_pool(name="nat", bufs=PF + 1))
        embT_pool = ctx.enter_context(tc.tile_pool(name="embT", bufs=PF + 2))
        exp_pool = ctx.enter_context(tc.tile_pool(name="exp", bufs=2))
        sm_pool = ctx.enter_context(tc.tile_pool(name="smalls", bufs=3))
        out_pool = ctx.enter_context(tc.tile_pool(name="outs", bufs=2))

        # PSUM (8 banks): P1 x2 + pA x2 + P3 x2 + smalls x2
        p1_pool = ctx.enter_context(tc.tile_pool(name="p1", bufs=2, space="PSUM"))
        pa_pool = ctx.enter_context(tc.tile_pool(name="pa", bufs=2, space="PSUM"))
        p3_pool = ctx.enter_context(tc.tile_pool(name="p3", bufs=2, space="PSUM"))
        ps_pool = ctx.enter_context(tc.tile_pool(name="ps", bufs=2, space="PSUM"))

        tiles = {}    # g -> (natsb, embTsb)
        state = {}    # g -> dict of live tiles

        def issue_load(g):
            eb = embT_pool.tile([E, G, NCH, CH], BF16, tag="embT")
            nc.gpsimd.dma_start(eb[:], embTg[g])
            nb = nat_pool.tile([CH, G, NCH, E], BF16, tag="nat")
            nc.gpsimd.dma_start(nb[:], natg[g])
            tiles[g] = (nb, eb)

        def emit_compat_half(g, half):
            if half == 0:
                natsb, embTsb = tiles.pop(g)
                P1 = p1_pool.tile([CH, G * H * NCH], F32, tag="p1")
                state[g] = dict(natsb=natsb, embTsb=embTsb, P1=P1)
            st = state[g]
            P1, embTsb = st["P1"], st["embTsb"]
            for i in range(half * 4, half * 4 + 4):
                u_sl = ug[:, (G * g + i) * H:(G * g + i + 1) * H]
                for c in range(NCH):
                    nc.tensor.matmul(
                        P1[:, G * H * c + H * i: G * H * c + H * i + H],
                        embTsb[:, i, c, :], u_sl, start=True, stop=True)
            if half == 1:
                expT = exp_pool.tile([CH, G * H * NCH], BF16, tag="expT")
                nc.scalar.activation(expT[:], P1[:], AF.Exp)
                st["expT"] = expT

        def emit_phaseA(g):
            st = state[g]
            expT, natsb = st["expT"], st["natsb"]
            sums_p = ps_pool.tile([1, G * H], F32, tag="ps")
            for c in range(NCH):
                nc.tensor.matmul(sums_p[:], ones1,
                                 expT[:, G * H * c: G * H * (c + 1)],
                                 start=(c == 0), stop=(c == NCH - 1))
            pA = pa_pool.tile([E, G * H], F32, tag="pa")
            for i in range(G):
                for c in range(NCH):
                    nc.tensor.matmul(
                        pA[:, H * i: H * i + H],
                        natsb[:, i, c, :],
                        expT[:, G * H * c + H * i: G * H * c + H * i + H],
                        start=(c == 0), stop=(c == NCH - 1))
            # 24 zero-padded rows in chunk 7 contribute exp(0)=1 each
            s64 = sm_pool.tile([1, G * H], F32, tag="s64")
            nc.vector.tensor_scalar_add(s64[:], sums_p[:], -24.0)
            st["pA"] = pA
            st["s64"] = s64

        def emit_phaseB1(g):
            # sums^T via tiny matmul, then reciprocal across 64 lanes
            st = state[g]
            sT_p = ps_pool.tile([G * H, 1], F32, tag="ps")
            nc.tensor.matmul(sT_p[:], st["s64"][:], onef, start=True, stop=True)
            rT = sm_pool.tile([G * H, 1], F32, tag="rT")
            nc.vector.reciprocal(rT[:], sT_p[:])
            rmat = sm_pool.tile([G * H, G], F32, tag="rmat")
            nc.vector.tensor_scalar_mul(rmat[:], imask, rT[:])
            st["rmat"] = rmat

        def emit_phaseB2(g):
            # R[hd, i] = r64[(i, hd//16)]; stage A out of psum
            st = state[g]
            R_p = ps_pool.tile([E, G], F32, tag="ps")
            nc.tensor.matmul(R_p[:], mask64, st["rmat"][:],
                             start=True, stop=True)
            R_sb = sm_pool.tile([E, G], F32, tag="R")
            nc.vector.tensor_scalar_add(R_sb[:], R_p[:], 0.0)
            A_sb = sm_pool.tile([E, G * H], F32, tag="A")
            nc.scalar.copy(A_sb[:], st["pA"][:])
            st["R_sb"] = R_sb
            st["A_sb"] = A_sb

        def emit_phaseB3(g):
            # heads = extract(Wvg.T @ A) * R
            st = state[g]
            pheads = ps_pool.tile([E, G * H], F32, tag="ps")
            nc.tensor.matmul(pheads[:], wvg, st["A_sb"][:],
                             start=True, stop=True)
            tmp = sm_pool.tile([E, G * H], F32, tag="tmp")
            nc.vector.tensor_mul(tmp[:], pheads[:], m8rep)
            heads8 = sm_pool.tile([E, G], F32, tag="heads8")
            nc.vector.reduce_sum(
                heads8[:], tmp[:].rearrange("p (i h) -> p i h", h=H),
                axis=AX.X)
            heads8r = sm_pool.tile([E, G], F32, tag="heads8r")
            nc.vector.tensor_mul(heads8r[:], heads8[:], st["R_sb"][:])
            st["heads8r"] = heads8r

        def emit_phaseB4(g):
            # w = Wbig.T @ heads
            st = state[g]
            pw = ps_pool.tile([E, G], F32, tag="ps")
            nc.tensor.matmul(pw[:], wbig, st["heads8r"][:],
                             start=True, stop=True)
            w8 = sm_pool.tile([E, G], BF16, tag="w8")
            nc.vector.tensor_scalar_add(w8[:], pw[:], 0.0)
            st["w8"] = w8

        def emit_phaseC(g):
            st = state.pop(g)
            embTsb, w8 = st["embTsb"], st["w8"]
            P3 = p3_pool.tile([CH, G * NCH], F32, tag="p3")
            for i in range(G):
                for c in range(NCH):
                    nc.tensor.matmul(P3[:, NCH * i + c: NCH * i + c + 1],
                                     embTsb[:, i, c, :], w8[:, i:i + 1],
                                     start=True, stop=True)
            c64 = sm_pool.tile([CH, G * NCH], F32, tag="c64")
            nc.scalar.copy(c64[:], P3[:])
            pt = ps_pool.tile([G * NCH, CH], F32, tag="ps")
            nc.tensor.transpose(pt[:], c64[:], identf)
            outsb = out_pool.tile([G * NCH, CH], F32, tag="outsb")
            nc.scalar.copy(outsb[:], pt[:])
            nc.sync.dma_start(outc[g], outsb[:])

        for g in range(min(PF, NG)):
            issue_load(g)

        # Software pipeline: group g-1's serial post-attention chain is
        # interleaved into group g's independent PE work so the DVE steps
        # always run under PE matmuls and the PE never drains.
        for g in range(NG):
            if g > 0:
                emit_phaseB1(g - 1)
            emit_compat_half(g, 0)
            if g > 0:
                emit_phaseB2(g - 1)
            emit_compat_half(g, 1)
            if g > 0:
                emit_phaseB3(g - 1)
                emit_phaseB4(g - 1)
                emit_phaseC(g - 1)
            if g + PF < NG:
                issue_load(g + PF)
            emit_phaseA(g)
        g = NG - 1
        emit_phaseB1(g)
        emit_phaseB2(g)
        emit_phaseB3(g)
        emit_phaseB4(g)
        emit_phaseC(g)

    nc.compile()
    return nc


def _get_nc():
    key = N_CORES
    if key not in _NC_CACHE:
        _NC_CACHE[key] = _build_nc(key)
    return _NC_CACHE[key]


def _make_const_blobs(Wvg, wbig):
    cf = np.zeros((E, F_COLS), np.float32)
    cf[:CH, F_IDENT:F_IDENT + CH] = np.eye(CH, dtype=np.float32)
    cf[:, F_WVG:F_WVG + E] = Wvg
    cf[:, F_WBIG:F_WBIG + E] = wbig
    m8 = np.zeros((E, G, H), np.float32)
    for hd in range(E):
        m8[hd, :, hd // D] = 1.0
    cf[:, F_M8:F_M8 + G * H] = m8.reshape(E, G * H)
    mk = np.zeros((G, H, E), np.float32)
    for hd in range(E):
        mk[:, hd // D, hd] = 1.0
    cf[:G * H, F_MASK64:F_MASK64 + E] = mk.reshape(G * H, E)
    im = np.zeros((G, H, G), np.float32)
    for i in range(G):
        im[i, :, i] = 1.0
    cf[:G * H, F_IMASK:F_IMASK + G] = im.reshape(G * H, G)
    cf[0, F_ONEF] = 1.0
    return np.ascontiguousarray(cf)


def kernel(embeddings, remaining_capacity, Wqg, Wkg, Wvg, Wog, Wqo, Wko,
           current_node, mask):
    global LAST_RESULT
    embeddings = np.asarray(embeddings, dtype=np.float32)
    remaining_capacity = np.asarray(remaining_capacity, dtype=np.float32)
    Wqg = np.asarray(Wqg, dtype=np.float32)
    Wkg = np.asarray(Wkg, dtype=np.float32)
    Wvg = np.asarray(Wvg, dtype=np.float32)
    Wog = np.asarray(Wog, dtype=np.float32)
    Wqo = np.asarray(Wqo, dtype=np.float32)
    Wko = np.asarray(Wko, dtype=np.float32)
    current_node = np.asarray(current_node)
    mask = np.asarray(mask)
    assert embeddings.shape == (B, N, E)

    trace = bool(os.environ.get("BASS_TRACE"))
    if trace:
        _install_profile_shim()

    # ---- host-side pre-attention chain ----
    graph = embeddings.mean(axis=1)                                # [B, E]
    cur = embeddings[np.arange(B), current_node.astype(np.int64)]  # [B, E]
    context = np.concatenate(
        [graph, cur, remaining_capacity[:, None]], axis=-1)        # [B, 257]
    q = (context @ Wqg).reshape(B, H, D)
    U = np.einsum('ehd,bhd->beh', Wkg.reshape(E, H, D), q) / math.sqrt(D)

    cf = _make_const_blobs(
        np.ascontiguousarray(Wvg).astype(np.float32),
        np.ascontiguousarray((Wog @ Wqo @ Wko.T) / math.sqrt(E)).astype(
            np.float32))

    embp = np.zeros((B, NPAD, E), dtype=BF16_NP)
    embp[:, :N] = embeddings.astype(BF16_NP)

    nc = _get_nc()
    in_maps = []
    for c in range(N_CORES):
        sl = slice(c * BC, (c + 1) * BC)
        t = embp[sl].reshape(NG, G, NCH, CH, E)
        natg = np.ascontiguousarray(t.transpose(0, 3, 1, 2, 4))
        embTg = np.ascontiguousarray(t.transpose(0, 4, 1, 2, 3))
        cb = np.zeros((E, B_COLS), BF16_NP)
        cb[:, :U_COLS] = U[sl].transpose(1, 0, 2).reshape(E, BC * H)
        cb[:CH, U_COLS] = 1.0
        m = {"natg": natg, "embTg": embTg, "cf": cf,
             "cb": np.ascontiguousarray(cb)}
        in_maps.append(m)

    kw = {}
    if trace:
        kw = dict(trace=True, trace_cores=[0])
    res = run_bass_kernel_spmd(nc, in_maps, list(range(N_CORES)), **kw)
    LAST_RESULT = res

    comp = np.concatenate(
        [res.results[c]["outc"].reshape(BC, NPAD)[:, :N]
         for c in range(N_CORES)], axis=0)

    logits = (10.0 * np.tanh(comp)).astype(np.float32)
    mx = logits.max(axis=-1, keepdims=True)
    ex = np.exp(logits - mx)
    probs = (ex / ex.sum(axis=-1, keepdims=True)).astype(np.float32)

    if mask.any():
        # General-correctness slow path (the spec always sends an all-False
        # mask): the mask affects the glimpse attention too, so recompute
        # everything for the masked rows on the host.
        probs, logits = _numpy_full(embeddings, remaining_capacity, Wqg, Wkg,
                                    Wvg, Wog, Wqo, Wko, cur, mask)

    return probs.astype(np.float32), logits.astype(np.float32)


def _numpy_full(emb, capv, Wqg, Wkg, Wvg, Wog, Wqo, Wko, cur, mask):
    graph = emb.mean(axis=1)
    context = np.concatenate([graph, cur, capv[:, None]], axis=-1)
    q = (context @ Wqg).reshape(B, H, D)
    k = (emb @ Wkg).reshape(B, N, H, D)
    v = (emb @ Wvg).reshape(B, N, H, D)
    compat = np.einsum('bhd,bnhd->bhn', q, k) / math.sqrt(D)
    compat = np.where(mask[:, None, :], -np.inf, compat)
    m = compat.max(axis=-1, keepdims=True)
    a = np.exp(compat - m)
    attn = a / a.sum(axis=-1, keepdims=True)
    heads = np.einsum('bhn,bnhd->bhd', attn, v).reshape(B, E)
    glimpse = heads @ Wog
    qo = glimpse @ Wqo
    ko = emb @ Wko
    comp = np.einsum('be,bne->bn', qo, ko) / math.sqrt(E)
    logits = 10.0 * np.tanh(comp)
    logits = np.where(mask, -np.inf, logits)
    m2 = logits.max(axis=-1, keepdims=True)
    a2 = np.exp(logits - m2)
    probs = a2 / a2.sum(axis=-1, keepdims=True)
    return probs.astype(np.float32), logits.astype(np.float32)


# revision 13
# speedup vs baseline: 1.8915x; 1.0814x over previous
"""Self-contained Trainium2 Bass kernel for nn_Decoder_79809082294812.

kernel(**inputs) takes the FULL unsharded inputs (embeddings [1024,1000,128],
remaining_capacity [1024], Wqg [257,128], Wkg/Wvg/Wog/Wqo/Wko [128,128],
current_node [1024], mask [1024,1000]) and returns (probs, logits), each
[1024, 1000] float32 - matching the reference decoder.

Sharding: pure data-parallel over the batch dim across 8 NeuronCores
(128 batch elements per core); weights replicated.

v3 design ("all-flipped, host-assisted, software-pipelined"):
  * The whole O(B*E^2) pre-attention chain (graph mean, context, q, U) is
    computed on the host; the device receives per-element glimpse key
    vectors U_b [E, H] directly.
  * compat is computed TRANSPOSED on-chip: for each (element i, n-chunk c)
    the stationary operand is an embT chunk [128e, 125n] and the moving
    operand is U_i [128, 8].  The psum result [125n, (c,i,h)] is already in
    "attn^T" orientation, so the PE transposes / psum->sbuf copies / attn
    normalization of the old kernel disappear.  No max subtraction before
    exp (|compat| < ~7).
  * The attn normalization (1/rowsum) is applied at the [128,8] heads
    stage via a tiny matmul-built broadcast.
  * comp (pre-tanh logits) is computed transposed too ([125n, (i,c)]
    psum), PE-transposed once per group, DMAed out as one contiguous
    block.  tanh / softmax / logits run on the host.
  * Embeddings ship in both layouts (nat + embT) as bf16, host-packed in
    group-blob layout so every DMA is a contiguous ~[128, 16KB] transfer.
    All constants ride in two single-DMA blobs ahead of the bulk.
  * Two-stage software pipeline: group g's post-attention chain (recip,
    heads, w, comp) is emitted after group g+1's compat so the PE queue
    always has independent work while DVE/ACT handle the serial steps.
"""
import contextlib
import ctypes
import math
import os
import sys
import types

sys.path.insert(0, '/opt/trn_rl_repo')

from contextlib import ExitStack
import numpy as np
import ml_dtypes

import concourse.bass as bass
import concourse.tile as tile
from concourse import bacc, mybir
from concourse.bass_utils import run_bass_kernel_spmd

F32 = mybir.dt.float32
BF16 = mybir.dt.bfloat16
AF = mybir.ActivationFunctionType
AX = mybir.AxisListType
ALU = mybir.AluOpType
BF16_NP = ml_dtypes.bfloat16

B = 1024
N = 1000
E = 128
H = 8
D = 16
N_CORES = 8
BC = B // N_CORES   # batch elements per core
G = 8               # elements per group
NG = BC // G        # 16 groups
NCH = 8             # n-chunks per element (chunk c = nodes [128c, 128c+128))
CH = 128            # chunk size (last chunk: 104 valid nodes + 24 zero pad)
NPAD = NCH * CH     # 1024
PF = 4              # DMA prefetch depth (groups)

# f32 const blob column layout
F_IDENT = 0      # [128,128] identity (for the output transpose)
F_WVG = 128      # [128,128]
F_WBIG = 256     # [128,128]
F_M8 = 384       # [128,64]
F_MASK64 = 448   # [64,128]
F_IMASK = 576    # [64,8]
F_ONEF = 584     # [1,1]
F_COLS = 585
# bf16 const blob: ug [128, BC*H] then ones column
U_COLS = BC * H
B_COLS = U_COLS + 1

_NC_CACHE = {}
LAST_RESULT = None   # BassKernelResults of the most recent run (for profiling)


# --------------------------------------------------------------------------
# Optional NTFF profiling hook (enabled only when BASS_TRACE is set).
# --------------------------------------------------------------------------
def _install_profile_shim():
    so_path = '/opt/axon/libaxon_pjrt.so'
    try:
        import antenv
    except ImportError:
        return
    if 'antenv.axon_hooks' not in sys.modules:
        mod = types.ModuleType('antenv.axon_hooks')
        mod._hook = None

        def set_axon_ntff_profile_hook(h):
            mod._hook = h

        def get_axon_ntff_profile_hook():
            return mod._hook

        mod.set_axon_ntff_profile_hook = set_axon_ntff_profile_hook
        mod.get_axon_ntff_profile_hook = get_axon_ntff_profile_hook
        sys.modules['antenv.axon_hooks'] = mod
        antenv.axon_hooks = mod
    mod = sys.modules['antenv.axon_hooks']
    if mod.get_axon_ntff_profile_hook() is not None:
        return
    try:
        lib = ctypes.CDLL(so_path)
    except OSError:
        return
    if not hasattr(lib, "axon_start_nrt_profile"):
        return
    lib.axon_start_nrt_profile.argtypes = [ctypes.POINTER(ctypes.c_int64),
                                           ctypes.c_size_t]
    lib.axon_start_nrt_profile.restype = ctypes.c_int64
    lib.axon_stop_nrt_profile.argtypes = [ctypes.c_char_p]
    lib.axon_stop_nrt_profile.restype = ctypes.c_int64

    @contextlib.contextmanager
    def _hook(output_dir, device_ids):
        import jax
        jax.devices()
        if device_ids:
            ids = (ctypes.c_int64 * len(device_ids))(*device_ids)
            rc = lib.axon_start_nrt_profile(ids, len(device_ids))
        else:
            rc = lib.axon_start_nrt_profile(None, 0)
        if rc != 0:
            raise RuntimeError(f"axon_start_nrt_profile rc={rc}")
        try:
            yield
        finally:
            n = lib.axon_stop_nrt_profile(str(output_dir).encode())
            if n < 0:
                raise RuntimeError(f"axon_stop_nrt_profile rc={n}")

    mod.set_axon_ntff_profile_hook(_hook)
    import concourse.bass_utils as bu
    bu.upload_artifacts = lambda tmpdir: f"local:{tmpdir}"


def _build_nc(n_devices=N_CORES):
    nc = bacc.Bacc("TRN2", target_bir_lowering=False, debug=False,
                   num_devices=n_devices)

    natg = nc.dram_tensor("natg", [NG, CH, G, NCH, E], BF16,
                          kind="ExternalInput").ap()
    embTg = nc.dram_tensor("embTg", [NG, E, G, NCH, CH], BF16,
                           kind="ExternalInput").ap()
    cf_d = nc.dram_tensor("cf", [E, F_COLS], F32, kind="ExternalInput").ap()
    cb_d = nc.dram_tensor("cb", [E, B_COLS], BF16, kind="ExternalInput").ap()
    outc = nc.dram_tensor("outc", [NG, G * NCH, CH], F32,
                          kind="ExternalOutput").ap()

    with tile.TileContext(nc) as tc, ExitStack() as ctx:
        # ---- constants: two blob DMAs on the (16-engine) gpsimd queue ----
        cpool = ctx.enter_context(tc.tile_pool(name="consts", bufs=1))
        cb = cpool.tile([E, B_COLS], BF16, tag="cb")
        nc.gpsimd.dma_start(cb[:], cb_d[:])
        cf = cpool.tile([E, F_COLS], F32, tag="cf")
        nc.gpsimd.dma_start(cf[:], cf_d[:])

        ug = cb[:, 0:U_COLS]
        ones1 = cb[:CH, U_COLS:U_COLS + 1]
        identf = cf[:CH, F_IDENT:F_IDENT + CH]
        wvg = cf[:, F_WVG:F_WVG + E]
        wbig = cf[:, F_WBIG:F_WBIG + E]
        m8rep = cf[:, F_M8:F_M8 + G * H]
        mask64 = cf[:G * H, F_MASK64:F_MASK64 + E]
        imask = cf[:G * H, F_IMASK:F_IMASK + G]
        onef = cf[:1, F_ONEF:F_ONEF + 1]

        # ---- pools ----
        nat_pool = ctx.enter_context(tc.tile_pool(name="nat", bufs=PF + 1))
        embT_pool = ctx.enter_context(tc.tile_pool(name="embT", bufs=PF + 2))
        exp_pool = ctx.enter_context(tc.tile_pool(name="exp", bufs=2))
        sm_pool = ctx.enter_context(tc.tile_pool(name="smalls", bufs=3))
        out_pool = ctx.enter_context(tc.tile_pool(name="outs", bufs=2))

        # PSUM (8 banks): P1 x2 + pA x2 + P3 x2 + smalls x2
        p1_pool = ctx.enter_context(tc.tile_pool(name="p1", bufs=2, space="PSUM"))
        pa_pool = ctx.enter_context(tc.tile_pool(name="pa", bufs=2, space="PSUM"))
        p3_pool = ctx.enter_context(tc.tile_pool(name="p3", bufs=2, space="PSUM"))
        ps_pool = ctx.enter_context(tc.tile_pool(name="ps", bufs=2, space="PSUM"))

        tiles = {}    # g -> (natsb, embTsb)
        state = {}    # g -> dict of live tiles

        def issue_load(g):
            eb = embT_pool.tile([E, G, NCH, CH], BF16, tag="embT")
            nc.gpsimd.dma_start(eb[:], embTg[g])
            nb = nat_pool.tile([CH, G, NCH, E], BF16, tag="nat")
            nc.gpsimd.dma_start(nb[:], natg[g])
            tiles[g] = (nb, eb)

        def emit_compat_half(g, half):
            # half 0 covers n-chunks c=0..3, half 1 covers c=4..7, each
            # followed by the matching exp() half so softmax overlaps compat.
            if half == 0:
                natsb, embTsb = tiles.pop(g)
                P1 = p1_pool.tile([CH, G * H * NCH], F32, tag="p1")
                expT = exp_pool.tile([CH, G * H * NCH], BF16, tag="expT")
                state[g] = dict(natsb=natsb, embTsb=embTsb, P1=P1, expT=expT)
            st = state[g]
            P1, embTsb, expT = st["P1"], st["embTsb"], st["expT"]
            lo, hi = half * 4 * G * H, (half + 1) * 4 * G * H
            for c in range(half * 4, half * 4 + 4):
                for i in range(G):
                    u_sl = ug[:, (G * g + i) * H:(G * g + i + 1) * H]
                    nc.tensor.matmul(
                        P1[:, G * H * c + H * i: G * H * c + H * i + H],
                        embTsb[:, i, c, :], u_sl, start=True, stop=True)
            nc.scalar.activation(expT[:, lo:hi], P1[:, lo:hi], AF.Exp)

        def emit_phaseA_half(g, half):
            st = state[g]
            expT, natsb = st["expT"], st["natsb"]
            if half == 0:
                sums_p = ps_pool.tile([1, G * H], F32, tag="ps", name="sums_p")
                for c in range(NCH):
                    nc.tensor.matmul(sums_p[:], ones1,
                                     expT[:, G * H * c: G * H * (c + 1)],
                                     start=(c == 0), stop=(c == NCH - 1))
                st["sums_p"] = sums_p
                st["pA"] = pa_pool.tile([E, G * H], F32, tag="pa", name="pA")
                # 24 zero-padded rows in chunk 7 contribute exp(0)=1 each
                s64 = sm_pool.tile([1, G * H], F32, tag="s64")
                nc.vector.tensor_scalar_add(s64[:], sums_p[:], -24.0)
                st["s64"] = s64
            # i-outer: each element's 8-chunk accumulation group stays
            # consecutive (start=True clears the bank's has_written bits, so
            # interleaving open groups in one bank corrupts them)
            pA = st["pA"]
            for i in range(half * 4, half * 4 + 4):
                for c in range(NCH):
                    nc.tensor.matmul(
                        pA[:, H * i: H * i + H],
                        natsb[:, i, c, :],
                        expT[:, G * H * c + H * i: G * H * c + H * i + H],
                        start=(c == 0), stop=(c == NCH - 1))

        def emit_phaseB1(g):
            # sums^T via tiny matmul, then reciprocal across 64 lanes
            st = state[g]
            sT_p = ps_pool.tile([G * H, 1], F32, tag="ps")
            nc.tensor.matmul(sT_p[:], st["s64"][:], onef, start=True, stop=True)
            rT = sm_pool.tile([G * H, 1], F32, tag="rT")
            nc.vector.reciprocal(rT[:], sT_p[:])
            rmat = sm_pool.tile([G * H, G], F32, tag="rmat")
            nc.vector.tensor_scalar_mul(rmat[:], imask, rT[:])
            st["rmat"] = rmat

        def emit_phaseB2(g):
            # R[hd, i] = r64[(i, hd//16)]; stage A out of psum
            st = state[g]
            R_p = ps_pool.tile([E, G], F32, tag="ps")
            nc.tensor.matmul(R_p[:], mask64, st["rmat"][:],
                             start=True, stop=True)
            R_sb = sm_pool.tile([E, G], F32, tag="R")
            nc.vector.tensor_scalar_add(R_sb[:], R_p[:], 0.0)
            A_sb = sm_pool.tile([E, G * H], F32, tag="A")
            nc.scalar.copy(A_sb[:], st["pA"][:])
            st["R_sb"] = R_sb
            st["A_sb"] = A_sb

        def emit_phaseB3(g):
            # heads = extract(Wvg.T @ A) * R
            st = state[g]
            pheads = ps_pool.tile([E, G * H], F32, tag="ps")
            nc.tensor.matmul(pheads[:], wvg, st["A_sb"][:],
                             start=True, stop=True)
            tmp = sm_pool.tile([E, G * H], F32, tag="tmp")
            nc.vector.tensor_mul(tmp[:], pheads[:], m8rep)
            heads8 = sm_pool.tile([E, G], F32, tag="heads8")
            nc.vector.reduce_sum(
                heads8[:], tmp[:].rearrange("p (i h) -> p i h", h=H),
                axis=AX.X)
            heads8r = sm_pool.tile([E, G], F32, tag="heads8r")
            nc.vector.tensor_mul(heads8r[:], heads8[:], st["R_sb"][:])
            st["heads8r"] = heads8r

        def emit_phaseB4(g):
            # w = Wbig.T @ heads
            st = state[g]
            pw = ps_pool.tile([E, G], F32, tag="ps")
            nc.tensor.matmul(pw[:], wbig, st["heads8r"][:],
                             start=True, stop=True)
            w8 = sm_pool.tile([E, G], BF16, tag="w8")
            nc.vector.tensor_scalar_add(w8[:], pw[:], 0.0)
            st["w8"] = w8

        def emit_phaseC1(g):
            st = state[g]
            embTsb, w8 = st["embTsb"], st["w8"]
            P3 = p3_pool.tile([CH, G * NCH], F32, tag="p3")
            for i in range(G):
                for c in range(NCH):
                    nc.tensor.matmul(P3[:, NCH * i + c: NCH * i + c + 1],
                                     embTsb[:, i, c, :], w8[:, i:i + 1],
                                     start=True, stop=True)
            c64 = sm_pool.tile([CH, G * NCH], F32, tag="c64")
            nc.vector.tensor_scalar_add(c64[:], P3[:], 0.0)
            st["c64"] = c64

        def emit_phaseC2(g):
            st = state.pop(g)
            pt = ps_pool.tile([G * NCH, CH], F32, tag="ps")
            nc.tensor.transpose(pt[:], st["c64"][:], identf)
            outsb = out_pool.tile([G * NCH, CH], F32, tag="outsb")
            nc.vector.tensor_scalar_add(outsb[:], pt[:], 0.0)
            nc.sync.dma_start(outc[g], outsb[:])

        for g in range(min(PF, NG)):
            issue_load(g)

        # Software pipeline: group g-1's serial post-attention chain is
        # interleaved into group g's independent PE work (compat/A halves)
        # so every DVE/ACT step runs under PE matmuls and the PE never
        # drains long enough to stall.
        for g in range(NG):
            if g > 0:
                emit_phaseB1(g - 1)
            emit_compat_half(g, 0)
            if g > 0:
                emit_phaseB2(g - 1)
            emit_compat_half(g, 1)
            if g > 0:
                emit_phaseB3(g - 1)
            emit_phaseA_half(g, 0)
            if g > 0:
                emit_phaseB4(g - 1)
            emit_phaseA_half(g, 1)
            if g > 0:
                emit_phaseC1(g - 1)
            if g + PF < NG:
                issue_load(g + PF)
            if g > 0:
                emit_phaseC2(g - 1)
        g = NG - 1
        emit_phaseB1(g)
        emit_phaseB2(g)
        emit_phaseB3(g)
        emit_phaseB4(g)
        emit_phaseC1(g)
        emit_phaseC2(g)

    nc.compile()
    return nc


def _get_nc():
    key = N_CORES
    if key not in _NC_CACHE:
        _NC_CACHE[key] = _build_nc(key)
    return _NC_CACHE[key]


def _make_const_blobs(Wvg, wbig):
    cf = np.zeros((E, F_COLS), np.float32)
    cf[:CH, F_IDENT:F_IDENT + CH] = np.eye(CH, dtype=np.float32)
    cf[:, F_WVG:F_WVG + E] = Wvg
    cf[:, F_WBIG:F_WBIG + E] = wbig
    m8 = np.zeros((E, G, H), np.float32)
    for hd in range(E):
        m8[hd, :, hd // D] = 1.0
    cf[:, F_M8:F_M8 + G * H] = m8.reshape(E, G * H)
    mk = np.zeros((G, H, E), np.float32)
    for hd in range(E):
        mk[:, hd // D, hd] = 1.0
    cf[:G * H, F_MASK64:F_MASK64 + E] = mk.reshape(G * H, E)
    im = np.zeros((G, H, G), np.float32)
    for i in range(G):
        im[i, :, i] = 1.0
    cf[:G * H, F_IMASK:F_IMASK + G] = im.reshape(G * H, G)
    cf[0, F_ONEF] = 1.0
    return np.ascontiguousarray(cf)


def kernel(embeddings, remaining_capacity, Wqg, Wkg, Wvg, Wog, Wqo, Wko,
           current_node, mask):
    global LAST_RESULT
    embeddings = np.asarray(embeddings, dtype=np.float32)
    remaining_capacity = np.asarray(remaining_capacity, dtype=np.float32)
    Wqg = np.asarray(Wqg, dtype=np.float32)
    Wkg = np.asarray(Wkg, dtype=np.float32)
    Wvg = np.asarray(Wvg, dtype=np.float32)
    Wog = np.asarray(Wog, dtype=np.float32)
    Wqo = np.asarray(Wqo, dtype=np.float32)
    Wko = np.asarray(Wko, dtype=np.float32)
    current_node = np.asarray(current_node)
    mask = np.asarray(mask)
    assert embeddings.shape == (B, N, E)

    trace = bool(os.environ.get("BASS_TRACE"))
    if trace:
        _install_profile_shim()

    # ---- host-side pre-attention chain ----
    graph = embeddings.mean(axis=1)                                # [B, E]
    cur = embeddings[np.arange(B), current_node.astype(np.int64)]  # [B, E]
    context = np.concatenate(
        [graph, cur, remaining_capacity[:, None]], axis=-1)        # [B, 257]
    q = (context @ Wqg).reshape(B, H, D)
    U = np.einsum('ehd,bhd->beh', Wkg.reshape(E, H, D), q) / math.sqrt(D)

    cf = _make_const_blobs(
        np.ascontiguousarray(Wvg).astype(np.float32),
        np.ascontiguousarray((Wog @ Wqo @ Wko.T) / math.sqrt(E)).astype(
            np.float32))

    embp = np.zeros((B, NPAD, E), dtype=BF16_NP)
    embp[:, :N] = embeddings.astype(BF16_NP)

    nc = _get_nc()
    in_maps = []
    for c in range(N_CORES):
        sl = slice(c * BC, (c + 1) * BC)
        t = embp[sl].reshape(NG, G, NCH, CH, E)
        natg = np.ascontiguousarray(t.transpose(0, 3, 1, 2, 4))
        embTg = np.ascontiguousarray(t.transpose(0, 4, 1, 2, 3))
        cb = np.zeros((E, B_COLS), BF16_NP)
        cb[:, :U_COLS] = U[sl].transpose(1, 0, 2).reshape(E, BC * H)
        cb[:CH, U_COLS] = 1.0
        m = {"natg": natg, "embTg": embTg, "cf": cf,
             "cb": np.ascontiguousarray(cb)}
        in_maps.append(m)

    kw = {}
    if trace:
        kw = dict(trace=True, trace_cores=[0])
    res = run_bass_kernel_spmd(nc, in_maps, list(range(N_CORES)), **kw)
    LAST_RESULT = res

    comp = np.concatenate(
        [res.results[c]["outc"].reshape(BC, NPAD)[:, :N]
         for c in range(N_CORES)], axis=0)

    logits = (10.0 * np.tanh(comp)).astype(np.float32)
    mx = logits.max(axis=-1, keepdims=True)
    ex = np.exp(logits - mx)
    probs = (ex / ex.sum(axis=-1, keepdims=True)).astype(np.float32)

    if mask.any():
        # General-correctness slow path (the spec always sends an all-False
        # mask): the mask affects the glimpse attention too, so recompute
        # everything for the masked rows on the host.
        probs, logits = _numpy_full(embeddings, remaining_capacity, Wqg, Wkg,
                                    Wvg, Wog, Wqo, Wko, cur, mask)

    return probs.astype(np.float32), logits.astype(np.float32)


def _numpy_full(emb, capv, Wqg, Wkg, Wvg, Wog, Wqo, Wko, cur, mask):
    graph = emb.mean(axis=1)
    context = np.concatenate([graph, cur, capv[:, None]], axis=-1)
    q = (context @ Wqg).reshape(B, H, D)
    k = (emb @ Wkg).reshape(B, N, H, D)
    v = (emb @ Wvg).reshape(B, N, H, D)
    compat = np.einsum('bhd,bnhd->bhn', q, k) / math.sqrt(D)
    compat = np.where(mask[:, None, :], -np.inf, compat)
    m = compat.max(axis=-1, keepdims=True)
    a = np.exp(compat - m)
    attn = a / a.sum(axis=-1, keepdims=True)
    heads = np.einsum('bhn,bnhd->bhd', attn, v).reshape(B, E)
    glimpse = heads @ Wog
    qo = glimpse @ Wqo
    ko = emb @ Wko
    comp = np.einsum('be,bne->bn', qo, ko) / math.sqrt(E)
    logits = 10.0 * np.tanh(comp)
    logits = np.where(mask, -np.inf, logits)
    m2 = logits.max(axis=-1, keepdims=True)
    a2 = np.exp(logits - m2)
    probs = a2 / a2.sum(axis=-1, keepdims=True)
    return probs.astype(np.float32), logits.astype(np.float32)
